# revision 44
# baseline (speedup 1.0000x reference)
"""Trainium2 Bass kernel for nn_MultiHeadAttention (dense transformer MHA).

Strategy (8-way tensor parallel over heads):
  - Each of the 8 cores owns 2 heads (128 of the 1024 q/k/v features).
  - Host pre-transposes the activations (query/key/value -> [D, T]) and casts
    to bf16; weights are head-sliced per core (Wo stays full).
  - The reference's RoPE variant uses neg_half = [y1, -y2] (not the usual
    rotate-half), which makes it a purely ELEMENTWISE transform:
        rope(y)[t, f] = y[t, f] * (cos(t*th_f) + sign_f * sin(t*th_f))
    so it is applied as one multiply by a host-precomputed factor C^T.
  - Attention is computed in the transposed layout S^T[s, t] so the P@V
    matmul needs no transposes.  Softmax is the "unsafe" variant (max |logit|
    ~ 10, exp is safe in fp32): exp on the Scalar engine, the denominator is
    obtained by appending a ones-column to V in the U = V'^T @ exp(S^T)
    matmul (row 64 of U accumulates colsum), and normalization is a
    partition-broadcast + elementwise multiply.
  - Re-partition from head-sharding to sequence-sharding with four AllToAll
    collectives (one per 512-wide t-chunk, overlapped with later chunks'
    compute via a chore scheduler that spreads deferred matmuls through the
    s-loop to avoid PE-queue head-of-line blocking); each core projects its
    own 64 rows per chunk through the full Wo and returns four [64, 1024]
    shards which the host concatenates.
"""
import numpy as np
import ml_dtypes

import concourse.bass as bass
import concourse.mybir as mybir
import concourse.tile as tile
from concourse import bacc
from concourse.bass_utils import run_bass_kernel_spmd

# problem constants (hardcoded per contract)
T = 2048
D = 1024
H = 16
DH = 64
ROPE_BASE = 10000

N_CORES = 8
HPC = H // N_CORES          # heads per core = 2
FPC = HPC * DH              # features per core = 128
TC = 512                    # attention t-chunk
NTC = T // TC               # 4
NS = T // 128               # 16 s-tiles
ND = D // 128               # 8 d-tiles
VW = 2 * DH + 2             # 130: v_ext block width per s-tile
ROWS = TC // N_CORES        # 64 output rows per core per A2A chunk

bf16 = mybir.dt.bfloat16
f32 = mybir.dt.float32
EXP = mybir.ActivationFunctionType.Exp

_cache = {}


def _build(use_bias=True):
    nc = bacc.Bacc("TRN2", target_bir_lowering=False, debug=False,
                   num_devices=N_CORES)

    # ---- I/O -----------------------------------------------------------
    qT = nc.dram_tensor("qT", [D, T], bf16, kind="ExternalInput").ap()
    kT = nc.dram_tensor("kT", [D, T], bf16, kind="ExternalInput").ap()
    vT = nc.dram_tensor("vT", [D, T], bf16, kind="ExternalInput").ap()
    wq = nc.dram_tensor("wq", [D, FPC], bf16, kind="ExternalInput").ap()
    wk = nc.dram_tensor("wk", [D, FPC], bf16, kind="ExternalInput").ap()
    wv = nc.dram_tensor("wv", [D, FPC], bf16, kind="ExternalInput").ap()
    wo = nc.dram_tensor("wo", [D, D], bf16, kind="ExternalInput").ap()
    bq = nc.dram_tensor("bq", [1, FPC], bf16, kind="ExternalInput").ap()
    bk = nc.dram_tensor("bk", [1, FPC], bf16, kind="ExternalInput").ap()
    bv = nc.dram_tensor("bv", [1, FPC], bf16, kind="ExternalInput").ap()
    bo = nc.dram_tensor("bo", [1, D], bf16, kind="ExternalInput").ap()
    ropeC = nc.dram_tensor("ropeC", [FPC, T], f32, kind="ExternalInput").ap()
    outs = [nc.dram_tensor(f"out{q}", [ROWS, D], f32,
                           kind="ExternalOutput").ap() for q in range(NTC)]

    with tile.TileContext(nc) as tc:
        with (
            tc.tile_pool(name="win", bufs=1) as win,        # weights/consts
            tc.tile_pool(name="xin", bufs=1) as xin,        # input stream
            tc.tile_pool(name="qk", bufs=NTC) as qkpool,    # q^T / k^T
            tc.tile_pool(name="vx", bufs=NS) as vxpool,     # v_ext
            tc.tile_pool(name="ex", bufs=8) as expool,      # exp(S^T)
            tc.tile_pool(name="at", bufs=1) as atpool,      # attn^T halves
            tc.tile_pool(name="nrm", bufs=4) as nrmpool,    # u_sb / Rbc
            tc.tile_pool(name="opr", bufs=2) as oprpool,    # out-proj tiles
            tc.tile_pool(name="pp", bufs=2, space="PSUM") as pproj,
            tc.tile_pool(name="ps", bufs=2, space="PSUM") as pS,
            tc.tile_pool(name="pu", bufs=2, space="PSUM") as pU,
            tc.tile_pool(name="dram", bufs=1, space="DRAM") as dram,
        ):
            # ---- constants / weights / inputs, in consumption order ----
            wq_sb = win.tile([128, ND * FPC], bf16, tag="wq")
            wk_sb = win.tile([128, ND * FPC], bf16, tag="wk")
            wv_sb = win.tile([128, ND * FPC], bf16, tag="wv")
            bq_sb = win.tile([1, FPC], bf16, tag="bq")
            bk_sb = win.tile([1, FPC], bf16, tag="bk")
            bv_sb = win.tile([1, FPC], bf16, tag="bv")
            bo_sb = win.tile([1, D], bf16, tag="bo")
            ropes = [win.tile([FPC, TC], f32, tag="rope", bufs=NTC,
                              name=f"rope{i}") for i in range(NTC)]
            ones_sb = win.tile([1, T], bf16, tag="ones")
            nc.gpsimd.memset(ones_sb[:], 1.0)
            onesf_sb = win.tile([1, DH], f32, tag="onesf")
            nc.gpsimd.memset(onesf_sb[:], 1.0)
            qin = xin.tile([128, ND * T], bf16, tag="qin")
            kin = xin.tile([128, ND * T], bf16, tag="kin")
            vin = xin.tile([128, ND * T], bf16, tag="vin")

            # q-side streams on the SP HWDGE ring, k-side on the ACT ring --
            # the two rings drain concurrently, halving the prologue.
            def _wdma(eng, w_sb, w):
                eng.dma_start(
                    out=w_sb[:].rearrange("p (d m) -> p d m", d=ND),
                    in_=w.rearrange("(d p) m -> p d m", p=128))

            def _xdma(eng, x_sb, x, d):
                eng.dma_start(
                    out=x_sb[:, T * d:T * (d + 1)],
                    in_=x[128 * d:128 * (d + 1), :])

            _wdma(nc.sync, wq_sb, wq)
            _wdma(nc.scalar, wk_sb, wk)
            nc.sync.dma_start(out=bq_sb[:], in_=bq)
            nc.scalar.dma_start(out=bk_sb[:], in_=bk)
            # chunk-0 rope first: the q0/k0 evictions must never wait on it
            nc.sync.dma_start(out=ropes[0][:], in_=ropeC[:, 0:TC])
            for d in range(ND):
                _xdma(nc.sync, qin, qT, d)
                _xdma(nc.scalar, kin, kT, d)
            for i in range(1, NTC):
                nc.scalar.dma_start(out=ropes[i][:],
                                    in_=ropeC[:, TC * i:TC * (i + 1)])
            _wdma(nc.sync, wv_sb, wv)
            nc.sync.dma_start(out=bv_sb[:], in_=bv)
            nc.sync.dma_start(out=bo_sb[:], in_=bo)
            for d in range(ND):
                _xdma(nc.sync if d % 2 == 0 else nc.scalar, vin, vT, d)
            wo_sb = win.tile([128, ND * D], bf16, tag="wo")
            nc.scalar.dma_start(
                out=wo_sb[:].rearrange("p (d m) -> p d m", d=ND),
                in_=wo.rearrange("(d p) m -> p d m", p=128))

            # PE warmup: ~16 back-to-back matmuls on the ones tile flip the
            # HAM clock gate to 8/8 while the input DMAs stream in.
            wup = pproj.tile([DH, 512], f32, tag="pp", name="wup")
            for _ in range(8):
                nc.tensor.matmul(wup[:], ones_sb[:, 0:DH], ones_sb[:, 0:512],
                                 start=True, stop=True)
            # consume the warmup result (it is exactly 1.0) so DCE keeps it
            nc.vector.tensor_copy(ones_sb[:, 0:512], wup[0:1, :])

            # ---- projections (per 512-wide chunk, chore-schedulable) ---
            qts = [qkpool.tile([128, TC], bf16, tag="qt", name=f"qt{i}")
                   for i in range(NTC)]
            kts = [qkpool.tile([128, TC], bf16, tag="kt", name=f"kt{i}")
                   for i in range(NTC)]

            def proj_chunk(which, tc_i):
                x_sb, w_sb, b_sb, x_in = {
                    "q": (qts[tc_i], wq_sb, bq_sb, qin),
                    "k": (kts[tc_i], wk_sb, bk_sb, kin),
                }[which]
                ts = slice(TC * tc_i, TC * (tc_i + 1))
                ps = pproj.tile([128, TC], f32, tag="pp",
                                name=f"pj_{which}{tc_i}")
                for d in range(ND):
                    nc.tensor.matmul(
                        ps[:], w_sb[:, FPC * d:FPC * (d + 1)],
                        x_in[:, T * d + TC * tc_i:T * d + TC * (tc_i + 1)],
                        start=(d == 0),
                        stop=(not use_bias and d == ND - 1))
                if use_bias:
                    nc.tensor.matmul(ps[:], b_sb[:], ones_sb[:, ts],
                                     start=False, stop=True)
                nc.vector.tensor_mul(x_sb[:], ps[:], ropes[tc_i][:])

            proj_chunk("q", 0)
            proj_chunk("k", 0)
            proj_chunk("k", 1)

            # v_ext: 16 tiles [128, VW]; block: [v_h0 | ones | v_h1 | ones]
            vs = [vxpool.tile([128, VW], bf16, tag="vext", name=f"vext{s}")
                  for s in range(NS)]
            for s in range(NS):
                nc.gpsimd.memset(vs[s][:, DH::DH + 1], 1.0)  # ones columns

            def vproj(s):
                ps = pproj.tile([128, FPC], f32, tag="pp", name=f"vps{s}")
                for d in range(ND):
                    nc.tensor.matmul(
                        ps[:], vin[:, T * d + 128 * s:T * d + 128 * (s + 1)],
                        wv_sb[:, FPC * d:FPC * (d + 1)],
                        start=(d == 0),
                        stop=(not use_bias and d == ND - 1))
                if use_bias:
                    nc.tensor.matmul(ps[:], ones_sb[:, 0:128], bv_sb[:],
                                     start=False, stop=True)
                nc.vector.tensor_copy(
                    vs[s][:].rearrange("p (h w) -> p h w", h=2)[:, :, 0:DH],
                    ps.rearrange("p (h w) -> p h w", h=2))

            # ---- attention + per-chunk A2A re-partition ---------------
            a2a_in = [dram.tile([8 * 128, ROWS], bf16, tag=f"a2ai{i}",
                                name=f"a2a_in{i}") for i in range(NTC)]
            a2a_out = [dram.tile([8 * 128, ROWS], bf16, tag=f"a2ao{i}",
                                 name=f"a2a_out{i}") for i in range(NTC)]

            # output projection for chunk q, split into 4 chore pieces so the
            # PE queue never bunches; state carried via dicts.
            ostate = {}

            def op1(q):
                ap = oprpool.tile([128, ND * ROWS], bf16, tag="aprj",
                                  name=f"aprj{q}")
                nc.sync.dma_start(
                    out=ap[:].rearrange("p (d t) -> p d t", d=ND),
                    in_=a2a_out[q].rearrange("(d p) t -> p d t", p=128))
                oev = oprpool.tile([ROWS, D], f32, tag="oev", name=f"oev{q}")
                ostate[q] = (ap, oev, [None, None])

            def _op_mms(q, n):
                ap, oev, po = ostate[q]
                po[n] = pproj.tile([ROWS, 512], f32, tag="pp",
                                   name=f"po{q}_{n}")
                nsl = slice(512 * n, 512 * (n + 1))
                for d in range(ND):
                    nc.tensor.matmul(
                        po[n][:], ap[:, ROWS * d:ROWS * (d + 1)],
                        wo_sb[:, D * d + 512 * n:D * d + 512 * (n + 1)],
                        start=(d == 0),
                        stop=(not use_bias and d == ND - 1))
                if use_bias:
                    nc.tensor.matmul(po[n][:], ones_sb[:, 0:ROWS],
                                     bo_sb[:, nsl], start=False, stop=True)

            def op2(q):
                _op_mms(q, 0)

            def op3(q):
                ap, oev, po = ostate[q]
                nc.vector.tensor_copy(oev[:, 0:512], po[0][:])
                _op_mms(q, 1)

            def op4(q):
                ap, oev, po = ostate[q]
                nc.vector.tensor_copy(oev[:, 512:1024], po[1][:])
                nc.sync.dma_start(out=outs[q], in_=oev[:])
                del ostate[q]

            # phase-b pieces: normalize chunk q's U (staged in SBUF) per
            # head, ship to the bounce, trigger the A2A
            nstate = {}

            def pb_h(q, h):
                u64, rr, aTs = nstate[q]
                rbp = pproj.tile([DH, TC], f32, tag="pp", name=f"rbp{q}_{h}")
                nc.tensor.matmul(rbp[:], onesf_sb[:], rr[h][:],
                                 start=True, stop=True)
                rbc = nrmpool.tile([DH, TC], f32, tag="rbc",
                                   name=f"rbc{q}_{h}")
                nc.vector.tensor_copy(rbc[:], rbp[:])
                aTs[h] = atpool.tile([DH, TC], bf16, tag=f"aT{h}",
                                     name=f"aTq{q}_{h}")
                nc.vector.tensor_mul(aTs[h][:], u64[h][:], rbc[:])

            def pb_ship(q):
                _, _, aTs = nstate[q]
                for h in range(HPC):
                    nc.sync.dma_start(
                        out=a2a_in[q].rearrange(
                            "(j h p) t -> h p j t", j=N_CORES, h=HPC)[h],
                        in_=aTs[h][:].rearrange("p (j t) -> p j t", j=N_CORES))
                nc.gpsimd.collective_compute(
                    "AllToAll", mybir.AluOpType.bypass,
                    replica_groups=[list(range(N_CORES))],
                    ins=[a2a_in[q][:].opt()],
                    outs=[a2a_out[q][:].opt()],
                )
                del nstate[q]

            for tc_i in range(NTC):
                ts = slice(TC * tc_i, TC * (tc_i + 1))
                # chores from earlier chunks, spread over this s-loop
                chores = []
                if tc_i == 0:
                    chores += [lambda: proj_chunk("k", 2),
                               lambda: proj_chunk("k", 3),
                               lambda: proj_chunk("q", 1),
                               lambda: proj_chunk("q", 2),
                               lambda: proj_chunk("q", 3)]
                if tc_i >= 1:
                    q = tc_i - 1
                    chores += [lambda q=q: pb_h(q, 0), lambda q=q: pb_h(q, 1),
                               lambda q=q: pb_ship(q)]
                if tc_i >= 2:
                    q = tc_i - 2
                    chores += [lambda q=q: op1(q), lambda q=q: op2(q),
                               lambda q=q: op3(q), lambda q=q: op4(q)]
                up = [pU.tile([DH + 1, TC], f32, tag="pu",
                              name=f"up{tc_i}_{h}") for h in range(HPC)]
                LAG = 4 if tc_i == 0 else 0
                exq = []

                def u_mms(s, ex):
                    for h in range(HPC):
                        o = (DH + 1) * h
                        nc.tensor.matmul(
                            up[h][:], vs[s][:, o:o + DH + 1],
                            ex[:, TC * h:TC * (h + 1)],
                            start=(s == 0), stop=(s == NS - 1))

                for s in range(NS):
                    if s >= 1 and chores:
                        chores.pop(0)()
                    kt_t = kts[s // 4]
                    ss = slice(128 * (s % 4), 128 * (s % 4 + 1))
                    sp = pS.tile([128, 2 * TC], f32, tag="ps")
                    nc.tensor.matmul(sp[:, 0:TC], kt_t[0:DH, ss],
                                     qts[tc_i][0:DH, :], start=True, stop=True)
                    nc.tensor.matmul(sp[:, TC:2 * TC], kt_t[DH:128, ss],
                                     qts[tc_i][DH:128, :], start=True,
                                     stop=True, tile_position=(DH, 0))
                    ex = expool.tile([128, 2 * TC], bf16, tag="ex")
                    nc.scalar.activation(ex[:], sp[:], EXP, scale=0.125)
                    exq.append(ex)
                    if s >= LAG:
                        sl = s - LAG
                        if tc_i == 0:
                            vproj(sl)
                        u_mms(sl, exq[sl])
                for sl in range(NS - LAG, NS):
                    if tc_i == 0:
                        vproj(sl)
                    u_mms(sl, exq[sl])
                for ch in chores:  # anything left (shouldn't happen)
                    ch()
                # phase A: stage U and 1/colsum to SBUF, freeing PSUM slots
                u64, rr = [], []
                for h in range(HPC):
                    u_sb = nrmpool.tile([DH, TC], f32, tag="u64",
                                        name=f"u64_{tc_i}_{h}")
                    nc.vector.tensor_copy(u_sb[:], up[h][0:DH, :])
                    r_sb = nrmpool.tile([1, TC], f32, tag="rsb",
                                        name=f"rsb{tc_i}_{h}")
                    nc.vector.tensor_copy(r_sb[:], up[h][DH:DH + 1, :])
                    nc.vector.reciprocal_approx_fast(r_sb[:], r_sb[:])
                    u64.append(u_sb)
                    rr.append(r_sb)
                nstate[tc_i] = (u64, rr, [None, None])
            # drain: chunks 2,3 phase-b/oproj not yet emitted
            pb_h(NTC - 1, 0), pb_h(NTC - 1, 1), pb_ship(NTC - 1)
            op1(NTC - 2), op2(NTC - 2), op3(NTC - 2), op4(NTC - 2)
            op1(NTC - 1), op2(NTC - 1), op3(NTC - 1), op4(NTC - 1)

    nc.compile()
    return nc


def _host_inputs(query, key, value, Wq, bq, Wk, bk, Wv, bv, Wo, bo):
    """Shard + lay out the full inputs for the 8 cores."""
    b = ml_dtypes.bfloat16
    qT = np.ascontiguousarray(query.T).astype(b)
    kT = np.ascontiguousarray(key.T).astype(b)
    vT = np.ascontiguousarray(value.T).astype(b)
    wo = Wo.astype(b)

    theta = 1.0 / (ROPE_BASE ** (np.arange(0, D, 2, dtype=np.float32) / D))
    idx = np.outer(np.arange(T, dtype=np.float32), theta)
    c, s = np.cos(idx), np.sin(idx)
    C = np.concatenate([c + s, c - s], axis=1).astype(np.float32)  # [T, D]

    in_maps = []
    for cidx in range(N_CORES):
        fs = slice(FPC * cidx, FPC * (cidx + 1))
        in_maps.append({
            "qT": qT, "kT": kT, "vT": vT,
            "wq": Wq[:, fs].astype(b), "wk": Wk[:, fs].astype(b),
            "wv": Wv[:, fs].astype(b), "wo": wo,
            "bq": bq[None, fs].astype(b), "bk": bk[None, fs].astype(b),
            "bv": bv[None, fs].astype(b), "bo": bo[None, :].astype(b),
            "ropeC": np.ascontiguousarray(C[:, fs].T),
        })
    return in_maps


def kernel(query, key, value, Wq, bq, Wk, bk, Wv, bv, Wo, bo, _trace=False):
    query, key, value = (np.asarray(x, np.float32) for x in (query, key, value))
    Wq, Wk, Wv, Wo = (np.asarray(x, np.float32) for x in (Wq, Wk, Wv, Wo))
    bq, bk, bv, bo = (np.asarray(x, np.float32) for x in (bq, bk, bv, bo))
    use_bias = any(np.any(b) for b in (bq, bk, bv, bo))
    ck = f"nc{int(use_bias)}"
    if ck not in _cache:
        _cache[ck] = _build(use_bias)
    nc = _cache[ck]
    in_maps = _host_inputs(query, key, value, Wq, bq, Wk, bk, Wv, bv, Wo, bo)
    res = run_bass_kernel_spmd(nc, in_maps, core_ids=list(range(N_CORES)),
                               trace=_trace)
    _cache["last_result"] = res
    out = np.empty((T, D), np.float32)
    for c in range(N_CORES):
        for q in range(NTC):
            r0 = TC * q + ROWS * c
            out[r0:r0 + ROWS, :] = res.results[c][f"out{q}"]
    return out
